# revision 19
# baseline (speedup 1.0000x reference)
"""Trainium2 Bass kernel for a CNF (Dormand-Prince 5(4), 10 steps) with
Hutchinson divergence, data-parallel over 8 NeuronCores.

kernel(**inputs) takes the FULL inputs from setup_inputs() and returns
(z [4096,64] fp32, log_px [4096,1] fp32), matching reference().
"""
import os
import sys

os.environ.setdefault("JAX_PLATFORMS", "axon,cpu")
for _p in ("/opt/trn_rl_repo", "/root/.axon_site/_ro/trn_rl_repo"):
    if _p not in sys.path:
        sys.path.insert(0, _p)

import numpy as np
import ml_dtypes

import concourse.bacc as bacc
import concourse.mybir as mybir
import concourse.tile as tile
from concourse import bass_utils

F32 = mybir.dt.float32
F32R = mybir.dt.float32r
BF16 = mybir.dt.bfloat16
AF = mybir.ActivationFunctionType
OP = mybir.AluOpType

T_FINAL = 1.0
NUM_STEPS = 10
DT = T_FINAL / NUM_STEPS
DP_C = [0.0, 1.0 / 5, 3.0 / 10, 4.0 / 5, 8.0 / 9, 1.0]
DP_A = [
    [],
    [1.0 / 5],
    [3.0 / 40, 9.0 / 40],
    [44.0 / 45, -56.0 / 15, 32.0 / 9],
    [19372.0 / 6561, -25360.0 / 2187, 64448.0 / 6561, -212.0 / 729],
    [9017.0 / 3168, -355.0 / 33, 46732.0 / 5247, 49.0 / 176, -5103.0 / 18656],
]
DP_B = [35.0 / 384, 0.0, 500.0 / 1113, 125.0 / 192, -2187.0 / 6784, 11.0 / 84]

N_CORES = 8
B_TOT, D, H = 4096, 64, 256
BC = B_TOT // N_CORES  # 512 batch per core

# RK combination coefficient table: one scaled 64x64 identity per (target
# stage, source stage) pair. Targets 1..5 use DP_A rows, target 6 (z_new)
# uses DP_B.
_COEF_ROWS = [[DT * a for a in DP_A[sp]] for sp in range(1, 6)] + [
    [DT * b for b in DP_B]
]
_IDS_OFFSETS = []
_ncoef = 0
for row in _COEF_ROWS:
    _IDS_OFFSETS.append(_ncoef)
    _ncoef += len(row)


def _build(num_steps):
    nc = bacc.Bacc("TRN2", target_bir_lowering=False, debug=False)

    def din(name, shape, dt=F32R):
        return nc.dram_tensor(name, shape, dt, kind="ExternalInput")

    YT = din("YT", [D, BC])
    ET = din("ET", [D, BC])
    W1ZC = din("W1ZC", [D, H])              # W1[:64,:]; lhsT for L1 fwd and for v
    W2C = din("W2C", [128, 2, H])           # W2 k-chunk major; lhsT for L2 fwd
    W2CBF = din("W2CBF", [128, 2, H], BF16)  # same, bf16 for the a2dot (JVP) matmuls
    W3C = din("W3C", [128, 2, D])           # W3 k-chunk major; lhsT for L3 fwd
    W3T = din("W3T", [D, H])                # W3.T; lhsT for u2
    W3S = din("W3S", [128, _ncoef, 2, D])   # coef-scaled W3 copies for RK combos
    B3C = din("B3C", [D, 6], F32)           # dt*sum(coefs)*b3 per target stage
    BIAS1 = din("BIAS1", [128, 2, 6 * num_steps], F32)  # t_s*W1[64,:] + b1, per stage
    BIAS2 = din("BIAS2", [128, 2], F32)     # b2 per chunk
    ONESB = din("ONESB", [128, 6], BF16)    # DP_B[s] broadcast, lhsT for acc reduce

    ZT_OUT = nc.dram_tensor("ZT", [D, BC], F32R, kind="ExternalOutput")
    ACC_OUT = nc.dram_tensor("ACC", [1, BC], F32, kind="ExternalOutput")

    with tile.TileContext(nc) as tc:
        with (
            tc.tile_pool(name="consts", bufs=1) as consts,
            tc.tile_pool(name="zpool", bufs=8) as zpool,
            tc.tile_pool(name="hpool", bufs=7) as hpool,
            tc.tile_pool(name="wpool", bufs=3) as wpool,
            tc.tile_pool(name="pbank", bufs=5, space="PSUM") as pbank,
            tc.tile_pool(name="padot", bufs=1, space="PSUM") as padot,
            tc.tile_pool(name="pacc", bufs=1, space="PSUM") as pacc,
        ):
            # ---- load constants ----
            def cload(dram, shape, dt=F32R):
                t = consts.tile(shape, dt, tag=dram.name)
                nc.sync.dma_start(out=t[...], in_=dram.ap())
                return t

            w1zc = cload(W1ZC, [D, H])
            w2c = cload(W2C, [128, 2, H])
            w2cbf = cload(W2CBF, [128, 2, H], BF16)
            w3c = cload(W3C, [128, 2, D])
            w3t = cload(W3T, [D, H])
            w3s = cload(W3S, [128, _ncoef, 2, D])
            b3c = cload(B3C, [D, 6], F32)
            bias1 = cload(BIAS1, [128, 2, 6 * num_steps], F32)
            bias2 = cload(BIAS2, [128, 2], F32)
            onesb = cload(ONESB, [128, 6], BF16)
            et = cload(ET, [D, BC])

            z0 = zpool.tile([D, BC], F32R, tag="z")
            nc.sync.dma_start(out=z0[...], in_=YT.ap())

            def bank():
                return pbank.tile([128, BC], F32, tag="bank", name="bank")

            # ---- prologue: u2 = W3 @ eT, v = (e @ W1z)^T  (bf16 copies) ----
            u2bf = consts.tile([128, 2, BC], BF16, tag="u2bf")
            vbf = consts.tile([128, 2, BC], BF16, tag="vbf")
            for mc in range(2):
                up = bank()
                nc.tensor.matmul(up[...], w3t[:, mc * 128:(mc + 1) * 128],
                                 et[...], start=True, stop=True)
                nc.vector.tensor_copy(u2bf[:, mc, :], up[...])
                vp = bank()
                nc.tensor.matmul(vp[...], w1zc[:, mc * 128:(mc + 1) * 128],
                                 et[...], start=True, stop=True)
                nc.vector.tensor_copy(vbf[:, mc, :], vp[...])

            accp = pacc.tile([1, BC], F32)
            n_acc = 2 * 6 * num_steps
            i_acc = 0

            # ---- main loop ----
            # Emission order = engine program order (engines run in-order).
            # The next-stage value znew is accumulated entirely in PSUM:
            #   znp = I*z0 + b3row + sum_{j<=s} (c_j*W3) @ h2_j
            # so no kz copies or partial-sum stts exist. The divergence path
            # of stage s is split across stages s+1 (squares/d/p ops) and
            # s+2 (adot matmuls, cdv, acc) so no engine queue ever stalls
            # ahead of the z critical chain.
            pend_a = None   # (h1, h2, s): stage awaiting sq/d/p ops
            pend_b = None   # (p1, p2, s): awaiting adot MMs + cdv
            pend_c = None   # (cdv, s):    awaiting the acc reduce MMs

            def emit_div_a(h1p, h2p, sp):
                sq1 = wpool.tile([128, 2, BC], BF16, tag="sq1", name="sq1")
                for mc in range(2):
                    nc.scalar.activation(sq1[:, mc, :], h1p[:, mc, :], AF.Square)
                d1 = wpool.tile([128, 2, BC], BF16, tag="d1", name="d1")
                p1 = wpool.tile([128, 2, BC], BF16, tag="p1", name="p1")
                for mc in range(2):
                    nc.vector.tensor_scalar(d1[:, mc, :], sq1[:, mc, :], 1.0,
                                            None, OP.subtract)
                    nc.vector.tensor_tensor(p1[:, mc, :], d1[:, mc, :],
                                            vbf[:, mc, :], OP.mult)
                sq2 = wpool.tile([128, 2, BC], BF16, tag="sq2", name="sq2")
                nc.gpsimd.tensor_tensor(sq2[...], h2p[...], h2p[...], OP.mult)
                d2 = wpool.tile([128, 2, BC], BF16, tag="d2", name="d2")
                p2 = wpool.tile([128, 2, BC], BF16, tag="p2", name="p2")
                for mc in range(2):
                    nc.vector.tensor_scalar(d2[:, mc, :], sq2[:, mc, :], 1.0,
                                            None, OP.subtract)
                    nc.vector.tensor_tensor(p2[:, mc, :], d2[:, mc, :],
                                            u2bf[:, mc, :], OP.mult)
                return (p1, p2, sp)

            def emit_div_b(p1, p2, sp):
                ad = padot.tile([128, 2, BC], F32, tag="adot", name="adot")
                for mc in range(2):
                    for kc in range(2):
                        nc.tensor.matmul(ad[:, mc, :],
                                         w2cbf[:, kc, mc * 128:(mc + 1) * 128],
                                         p1[:, kc, :],
                                         start=(kc == 0), stop=(kc == 1))
                cdv = wpool.tile([128, 2, BC], BF16, tag="cdv", name="cdv")
                for mc in range(2):
                    nc.vector.tensor_tensor(cdv[:, mc, :], p2[:, mc, :],
                                            ad[:, mc, :], OP.mult)
                return (cdv, sp)

            def emit_div_c(cdv, sp):
                nonlocal i_acc
                for mc in range(2):
                    i_acc += 1
                    nc.tensor.matmul(accp[...], onesb[:, sp:sp + 1],
                                     cdv[:, mc, :],
                                     start=(i_acc == 1), stop=(i_acc == n_acc))

            for n in range(num_steps):
                h2s = []
                for s in range(6):
                    zin = z0 if s == 0 else zs_next  # noqa: F821
                    col = 6 * n + s
                    tgt = s
                    # znew accumulator: earlier-stage W3 terms
                    znp = bank()
                    for j in range(s):
                        i = _IDS_OFFSETS[tgt] + j
                        for kc in range(2):
                            nc.tensor.matmul(znp[0:D, :], w3s[:, i, kc, :],
                                             h2s[j][:, kc, :],
                                             start=(j == 0 and kc == 0),
                                             stop=False)
                    # L1 + tanh1
                    h1 = hpool.tile([128, 2, BC], F32R, tag="h1")
                    for mc in range(2):
                        a1 = bank()
                        nc.tensor.matmul(a1[...],
                                         w1zc[:, mc * 128:(mc + 1) * 128],
                                         zin[...], start=True, stop=True)
                        nc.scalar.activation(h1[:, mc, :], a1[...], AF.Tanh,
                                             bias=bias1[:, mc, col:col + 1])
                    # divergence adot+cdv of stage s-2: PE work fills the
                    # tanh1 wait; inputs (p1, p2) were finished last stage
                    if pend_b is not None:
                        pend_c_new = emit_div_b(*pend_b)
                        pend_b = None
                    else:
                        pend_c_new = None
                    # L2 + tanh2 (kc-major so the first MMs need only h1c0)
                    h2 = hpool.tile([128, 2, BC], F32R, tag="h2")
                    a2c = [bank(), bank()]
                    for kc in range(2):
                        for mc in range(2):
                            nc.tensor.matmul(a2c[mc][...],
                                             w2c[:, kc, mc * 128:(mc + 1) * 128],
                                             h1[:, kc, :],
                                             start=(kc == 0), stop=(kc == 1))
                    # acc reduce of stage s-3 (cdv long since ready)
                    if pend_c is not None:
                        emit_div_c(*pend_c)
                    pend_c = pend_c_new
                    i = _IDS_OFFSETS[tgt] + s
                    for mc in range(2):
                        nc.scalar.activation(h2[:, mc, :], a2c[mc][...], AF.Tanh,
                                             bias=bias2[:, mc:mc + 1])
                        # final RK term for this h2 chunk right after its tanh
                        nc.tensor.matmul(znp[0:D, :], w3s[:, i, mc, :],
                                         h2[:, mc, :],
                                         start=(s == 0 and mc == 0),
                                         stop=(mc == 1))
                    h2s.append(h2)
                    znew = zpool.tile([D, BC], F32R, tag="z")
                    nc.vector.scalar_tensor_tensor(znew[...], znp[0:D, :],
                                                   b3c[:, tgt:tgt + 1],
                                                   z0[...], OP.add, OP.add)
                    if s < 5:
                        zs_next = znew
                    else:
                        z0 = znew
                    # squares/d/p of stage s-1 (emitted last so the z chain
                    # stays ahead in the ACT/DVE queues)
                    if pend_a is not None:
                        pend_b = emit_div_a(*pend_a)
                        pend_a = None
                    pend_a = (h1, h2, s)

            # drain the divergence pipeline
            if pend_c is not None:
                emit_div_c(*pend_c)
                pend_c = None
            pb2 = emit_div_a(*pend_a)
            emit_div_c(*emit_div_b(*pend_b))
            emit_div_c(*emit_div_b(*pb2))

            acc_sb = consts.tile([1, BC], F32, tag="acc_sb")
            nc.vector.tensor_copy(acc_sb[...], accp[...])
            nc.sync.dma_start(out=ZT_OUT.ap(), in_=z0[...])
            nc.sync.dma_start(out=ACC_OUT.ap(), in_=acc_sb[...])

    nc.compile()
    return nc


_NC_CACHE = {}


def _get_nc(num_steps=NUM_STEPS):
    if num_steps not in _NC_CACHE:
        _NC_CACHE[num_steps] = _build(num_steps)
    return _NC_CACHE[num_steps]


def _make_in_maps(y, e, W1, b1, W2, b2, W3, b3, num_steps=NUM_STEPS):
    f = np.float32
    y = np.asarray(y, f)
    e = np.asarray(e, f)
    W1 = np.asarray(W1, f)
    b1 = np.asarray(b1, f)
    W2 = np.asarray(W2, f)
    b2 = np.asarray(b2, f)
    W3 = np.asarray(W3, f)
    b3 = np.asarray(b3, f)

    w1zc = np.ascontiguousarray(W1[:D, :])
    w1t = W1[D, :]
    w2c = np.ascontiguousarray(W2.reshape(2, 128, H).transpose(1, 0, 2))
    w3c = np.ascontiguousarray(W3.reshape(2, 128, D).transpose(1, 0, 2))
    w3t = np.ascontiguousarray(W3.T)

    # per-stage L1 bias: t_s * W1[64,:] + b1, laid out [128, chunk, stage]
    cols = []
    for n in range(num_steps):
        for s in range(6):
            t_s = (n + DP_C[s]) * DT
            cols.append(t_s * w1t + b1)
    bias1 = np.stack(cols, axis=0)  # [6*steps, 256]
    bias1 = np.ascontiguousarray(bias1.reshape(-1, 2, 128).transpose(2, 1, 0))
    bias2 = np.ascontiguousarray(b2.reshape(2, 128).T)

    w3s = np.zeros((128, _ncoef, 2, D), f)
    i = 0
    for row in _COEF_ROWS:
        for cval in row:
            w3s[:, i, :, :] = cval * w3c
            i += 1
    b3c = np.zeros((D, 6), f)
    for tgt, row in enumerate(_COEF_ROWS):
        b3c[:, tgt] = sum(row) * b3

    onesb = np.zeros((128, 6), ml_dtypes.bfloat16)
    for s in range(6):
        onesb[:, s] = DP_B[s]

    common = {
        "W1ZC": w1zc, "W2C": w2c, "W2CBF": w2c.astype(ml_dtypes.bfloat16),
        "W3C": w3c, "W3T": w3t,
        "W3S": w3s, "B3C": b3c,
        "BIAS1": bias1, "BIAS2": bias2, "ONESB": onesb,
    }
    in_maps = []
    for c in range(N_CORES):
        sl = slice(c * BC, (c + 1) * BC)
        in_maps.append({
            **common,
            "YT": np.ascontiguousarray(y[sl].T),
            "ET": np.ascontiguousarray(e[sl].T),
        })
    return in_maps


def _postprocess(results):
    zs, lps = [], []
    for c in range(N_CORES):
        zt = results[c]["ZT"]          # [64, 512]
        acc = results[c]["ACC"][0]     # [512]
        z = np.ascontiguousarray(zt.T)  # [512, 64]
        log_pz = np.sum(-0.5 * np.log(2.0 * np.pi) - 0.5 * z * z,
                        axis=1, keepdims=True).astype(np.float32)
        log_px = log_pz + (DT * acc)[:, None].astype(np.float32)
        zs.append(z)
        lps.append(log_px)
    return np.concatenate(zs, axis=0), np.concatenate(lps, axis=0).astype(np.float32)


def run_on_cores(num_steps, **inputs):
    nc = _get_nc(num_steps)
    in_maps = _make_in_maps(num_steps=num_steps, **inputs)
    res = bass_utils.run_bass_kernel_spmd(nc, in_maps, core_ids=list(range(N_CORES)))
    return res.results


def kernel(y, e, W1, b1, W2, b2, W3, b3):
    try:
        results = run_on_cores(NUM_STEPS, y=y, e=e, W1=W1, b1=b1, W2=W2,
                               b2=b2, W3=W3, b3=b3)
    except Exception:
        # one retry for transient device errors
        results = run_on_cores(NUM_STEPS, y=y, e=e, W1=W1, b1=b1, W2=W2,
                               b2=b2, W3=W3, b3=b3)
    return _postprocess(results)
